# revision 1
# baseline (speedup 1.0000x reference)
"""Trainium2 Bass kernel for nn_DEMFeatureGenerator.

Input  x: [8, 3, 1024, 1024] fp32 (NCHW).
Output:   [8, 6, 1024, 1024] fp32 — 6 per-sample-min-max-normalized DEM features.

Strategy (pure data parallel, 1 image per NeuronCore, 8 cores):
 - Host: scale-by-255 decision (global max), RGB->gray (exact reference fp32
   arithmetic order), transpose each image to T-layout [cols, rows].
 - Device (per core, T-layout: partition dim = image columns, free dim = rows):
   9 column tiles (K=128 input cols with col-halo; M=112/120 output cols).
   All convolutions run on the TensorEngine as PSUM-accumulated banded matmuls:
   the horizontal (column) kernel lives in a banded lhsT matrix (reflect
   padding folded into edge-tile bands); the vertical (row) kernel becomes
   free-dim shifts of the rhs with reflect-padded SBUF tiles.
   Features are stored pre-normalization in SBUF as fp16 (sqrt deferred via
   monotonicity for slope/roughness), min/max reduced on the fly, then a
   second on-chip pass applies the (x-min)/(max-min) affine and streams out.
"""
import math

import numpy as np

import concourse.bacc as bacc
import concourse.bass as bass
import concourse.bass_isa as bass_isa
import concourse.mybir as mybir
import concourse.tile as tile
from concourse import bass_utils

F32 = mybir.dt.float32
F16 = mybir.dt.float16
OP = mybir.AluOpType
AF = mybir.ActivationFunctionType

NCOLS = 1024
NROWS = 1024
NCHUNK = 2
CHUNK = NROWS // NCHUNK  # 512

# (in_c0, K, out_c0, M); g_c0 = max(out_c0 - 7, 0), Mg = min(out+M+7,1024)-g_c0
TILES = [(0, 128, 0, 120)] + [(112 * t, 128, 112 * t + 8, 112) for t in range(1, 8)] + [(896, 128, 904, 120)]

S2_SCALE = float(2.0 ** -12)          # slope^2 fp16 overflow guard (positive scale
                                      # is invariant under min-max normalization)
EPS_RECIP = 1e-30                     # sx==0 guard for atan2 division


def _gaussian_kernel_2d(ksize=3, sigma=0.0):
    if sigma <= 0:
        sigma = 0.3 * ((ksize - 1) * 0.5 - 1) + 0.8
    center = ksize // 2
    xs = np.arange(ksize, dtype=np.float32) - center
    g1 = np.exp(-xs ** 2 / (2 * sigma ** 2))
    g2 = g1[None, :] * g1[:, None]
    return (g2 / g2.sum()).astype(np.float32)


def _refl(i, n=NCOLS):
    if i < 0:
        return -i
    if i >= n:
        return 2 * (n - 1) - i
    return i


def _g_cols(in_c0, K, out_c0, M):
    """Permuted g-tile column order: output cols first (psum partitions 0..M-1),
    then left halo, then right halo. Keeps every compute-engine access
    partition-0-aligned (HW requires start partition in {0,32,64,96})."""
    g_c0 = max(out_c0 - 7, 0)
    g_end = min(out_c0 + M + 7, NCOLS)
    cols = list(range(out_c0, out_c0 + M))
    cols += list(range(g_c0, out_c0))
    cols += list(range(out_c0 + M, g_end))
    return cols


def _make_band_g1(in_c0, K, g_cols, taps):
    """Gauss band: input x cols (natural order) -> permuted g cols."""
    B = np.zeros((K, len(g_cols)), np.float32)
    for m, c_out in enumerate(g_cols):
        for d, w in taps.items():
            k = _refl(c_out + d) - in_c0
            assert 0 <= k < K
            B[k, m] += np.float32(w)
    return B


def _make_band2(g_cols, out_c0, M, taps):
    """Second-level band: permuted g cols -> output cols 0..M-1."""
    pos = {c: i for i, c in enumerate(g_cols)}
    B = np.zeros((len(g_cols), M), np.float32)
    for m in range(M):
        for d, w in taps.items():
            k = pos[_refl(out_c0 + m + d)]
            B[k, m] += np.float32(w)
    return B


def _tile_geom(in_c0, K, out_c0, M):
    g_c0 = max(out_c0 - 7, 0)
    g_end = min(out_c0 + M + 7, NCOLS)
    return g_c0, g_end - g_c0


def build_bands():
    """Concatenate all lhsT band matrices into one [128, TOT] fp32 array.
    Returns (array, offsets) where offsets[(set_idx, name)] = (col_off, K, M).
    set_idx: 0 = tile 0, 1 = tiles 1..7 (shared), 2 = tile 8."""
    K2 = _gaussian_kernel_2d()  # [ky, kx] exact reference weights
    cols = []
    offsets = {}
    off = 0

    def add(name, set_idx, B):
        nonlocal off
        K, M = B.shape
        pad = np.zeros((128, 128), np.float32)
        pad[:K, :M] = B
        cols.append(pad)
        offsets[(set_idx, name)] = (off, K, M)
        off += 128

    for set_idx, tidx in ((0, 0), (1, 1), (2, 8)):
        in_c0, K, out_c0, M = TILES[tidx]
        gc = _g_cols(in_c0, K, out_c0, M)
        # gauss bands (one per ky, exact 2D weights incl. fold of reflect)
        for ky in range(3):
            add(f"bg{ky}", set_idx,
                _make_band_g1(in_c0, K, gc, {-1: K2[ky, 0], 0: K2[ky, 1], 1: K2[ky, 2]}))
        # sobel_x: diff_h band, x1 and x2 scaled ([1,2,1] vertical smooth)
        bdh = _make_band2(gc, out_c0, M, {-1: -1.0, 1: 1.0})
        add("bdh1", set_idx, bdh)
        add("bdh2", set_idx, 2.0 * bdh)
        # sobel_y: smooth_h band, +/- ([-1,0,1] vertical diff)
        bsh = _make_band2(gc, out_c0, M, {-1: 1.0, 0: 2.0, 1: 1.0})
        add("bshp", set_idx, bsh)
        add("bshm", set_idx, -bsh)
        # laplacian [[2,0,2],[0,-8,0],[2,0,2]]
        add("bph", set_idx, _make_band2(gc, out_c0, M, {-1: 2.0, 1: 2.0}))
        add("bcm8", set_idx, _make_band2(gc, out_c0, M, {0: -8.0}))
        # box5 (unnormalized; 1/25 folded into pointwise)
        add("bb5", set_idx, _make_band2(gc, out_c0, M, {d: 1.0 for d in range(-2, 3)}))
        # box15 horizontal with 1/225 folded
        add("bb15", set_idx, _make_band2(gc, out_c0, M, {d: 1.0 / 225.0 for d in range(-7, 8)}))
        # -identity [M, M] (ld stage-B shifts of t1)
        add("bitm", set_idx, -np.eye(M, dtype=np.float32))

    return np.concatenate(cols, axis=1), offsets


_NC_CACHE = {}
PERF_ABLATE = set()  # {'mm','pw','phase2','evac','reduce'} - timing experiments only


def build_nc():
    abl = PERF_ABLATE
    bands_np, boff = build_bands()
    TOT = bands_np.shape[1]

    nc = bacc.Bacc("TRN2", target_bir_lowering=False, debug=False, num_devices=8)
    gray_d = nc.dram_tensor("gray", [NCOLS, NROWS], F32, kind="ExternalInput")
    bands_d = nc.dram_tensor("bands", [128, TOT], F32, kind="ExternalInput")
    out_d = nc.dram_tensor("out", [6, NCOLS, NROWS], F32, kind="ExternalOutput")

    with tile.TileContext(nc) as tc:
        with (
            tc.tile_pool(name="const", bufs=1) as constp,
            tc.tile_pool(name="feat", bufs=1) as featp,
            tc.tile_pool(name="gray", bufs=2) as grayp,
            tc.tile_pool(name="gsb", bufs=2) as gsbp,
            tc.tile_pool(name="g2sb", bufs=2) as g2sbp,
            tc.tile_pool(name="t1sb", bufs=2) as t1sbp,
            tc.tile_pool(name="ptmp", bufs=10) as ptp,
            tc.tile_pool(name="stage", bufs=3) as stagep,
            tc.tile_pool(name="psum", bufs=8, space="PSUM") as psp,
        ):
            bsb = constp.tile([128, TOT], F32)
            nc.sync.dma_start(bsb[:], bands_d[:])

            zeros = constp.tile([128, 1], F32)
            nc.gpsimd.memset(zeros[:], 0.0)

            # per-feature, per-chunk min/max accumulators [128, 18]
            accmn = [constp.tile([128, 9], F32, name=f"accmn{f}") for f in range(6)]
            accmx = [constp.tile([128, 9], F32, name=f"accmx{f}") for f in range(6)]
            for f in range(6):
                nc.gpsimd.memset(accmn[f][:], 3.0e38)
                nc.gpsimd.memset(accmx[f][:], -3.0e38)

            featbuf = featp.tile([128, 6 * 9 * 1024], F16, name="featbuf")
            feats = [featbuf[:, 9216 * f: 9216 * (f + 1)] for f in range(6)]

            def band(name, set_idx):
                o, K, M = boff[(set_idx, name)]
                return bsb[:, o:o + 128]

            # ---------------- phase 1 ----------------
            for tidx, (in_c0, K, out_c0, M) in enumerate(TILES):
                s = 0 if tidx == 0 else (2 if tidx == 8 else 1)
                g_c0, Mg = _tile_geom(in_c0, K, out_c0, M)
                po = 0                        # out cols at partitions 0..M-1 (permuted bands)
                fcol = 1024 * tidx            # feature-store column base for this tile

                gt = grayp.tile([128, 1 + NROWS + 1], F32, tag="gray")
                nc.sync.dma_start(gt[:, 1:1 + NROWS], gray_d[in_c0:in_c0 + K, :])
                nc.vector.tensor_copy(gt[:, 0:1], gt[:, 2:3])
                nc.vector.tensor_copy(gt[:, 1 + NROWS:2 + NROWS], gt[:, NROWS - 1:NROWS])

                g_sb = gsbp.tile([128, 7 + NROWS + 7], F32, tag="g")
                g2_sb = g2sbp.tile([128, 7 + NROWS + 7], F32, tag="g2")

                # --- gauss: psum_g = sum_ky band(bg_ky) @ gray[:, chunk+ky-1] ---
                for c in range(NCHUNK):
                    pg = psp.tile([128, CHUNK], F32, tag="ps")
                    for i, ky in enumerate((0, 1, 2)):
                        nc.tensor.matmul(
                            pg[:, :], band(f"bg{ky}", s),
                            gt[:, c * CHUNK + ky: c * CHUNK + ky + CHUNK],
                            start=(i == 0), stop=(i == 2))
                    nc.scalar.activation(g_sb[:, 7 + c * CHUNK: 7 + (c + 1) * CHUNK],
                                         pg[:, :], AF.Copy)
                    nc.scalar.activation(g2_sb[:, 7 + c * CHUNK: 7 + (c + 1) * CHUNK],
                                         pg[:, :], AF.Square)
                    # elevation feature store (fp16)
                    nc.vector.tensor_copy(
                        feats[0][0:M, fcol + c * CHUNK: fcol + (c + 1) * CHUNK],
                        pg[0:M, :])

                # reflect y-pads for g, g2
                for tt in (g_sb, g2_sb):
                    nc.vector.tensor_copy(tt[:, 0:7], tt[:, 14:7:-1])
                    nc.vector.tensor_copy(tt[:, 7 + NROWS:14 + NROWS],
                                          tt[:, 5 + NROWS:NROWS - 2:-1])

                t1_sb = t1sbp.tile([128, 6 + NROWS + 6], F32, tag="t1")

                for c in range(NCHUNK):
                    ci = 2 * tidx + c
                    lo = 7 + c * CHUNK  # base offset of this chunk in g_sb coords

                    def gs(dy, src=g_sb):
                        return src[:, lo + dy: lo + dy + CHUNK]

                    fsl = lambda f: feats[f][0:M, fcol + c * CHUNK: fcol + (c + 1) * CHUNK]
                    zb = zeros[0:M, 0:1].broadcast_to((M, CHUNK))

                    # --- sobel x ---
                    psx = psp.tile([128, CHUNK], F32, tag="ps")
                    nc.tensor.matmul(psx[:, :], band("bdh1", s), gs(-1), start=True, stop=False)
                    nc.tensor.matmul(psx[:, :], band("bdh2", s), gs(0), start=False, stop=False)
                    nc.tensor.matmul(psx[:, :], band("bdh1", s), gs(1), start=False, stop=True)
                    # --- sobel y ---
                    psy = psp.tile([128, CHUNK], F32, tag="ps")
                    nc.tensor.matmul(psy[:, :], band("bshm", s), gs(-1), start=True, stop=False)
                    nc.tensor.matmul(psy[:, :], band("bshp", s), gs(1), start=False, stop=True)
                    # --- curvature ---
                    pcv = psp.tile([128, CHUNK], F32, tag="ps")
                    nc.tensor.matmul(pcv[:, :], band("bph", s), gs(-1), start=True, stop=False)
                    nc.tensor.matmul(pcv[:, :], band("bcm8", s), gs(0), start=False, stop=False)
                    nc.tensor.matmul(pcv[:, :], band("bph", s), gs(1), start=False, stop=True)
                    # --- box5(g), box5(g^2) ---
                    pb5 = psp.tile([128, CHUNK], F32, tag="ps")
                    for i, dy in enumerate(range(-2, 3)):
                        nc.tensor.matmul(pb5[:, :], band("bb5", s), gs(dy),
                                         start=(i == 0), stop=(i == 4))
                    pb52 = psp.tile([128, CHUNK], F32, tag="ps")
                    for i, dy in enumerate(range(-2, 3)):
                        nc.tensor.matmul(pb52[:, :], band("bb5", s), gs(dy, g2_sb),
                                         start=(i == 0), stop=(i == 4))
                    # --- box15 stage A ---
                    pt1 = psp.tile([128, CHUNK], F32, tag="ps")
                    for i, dy in enumerate((-1, 0, 1)):
                        nc.tensor.matmul(pt1[:, :], band("bb15", s), gs(dy),
                                         start=(i == 0), stop=(i == 2))
                    nc.scalar.activation(t1_sb[:, 6 + c * CHUNK: 6 + (c + 1) * CHUNK],
                                         pt1[:, :], AF.Copy)

                    # ---- pointwise ----
                    sx2 = ptp.tile([128, CHUNK], F32, tag="pw", name="pw_sx2")
                    nc.scalar.activation(sx2[0:M, :], psx[0:M, :], AF.Square, scale=2.0 ** -6)
                    sy2 = ptp.tile([128, CHUNK], F32, tag="pw", name="pw_sy2")
                    nc.scalar.activation(sy2[0:M, :], psy[0:M, :], AF.Square, scale=2.0 ** -6)
                    sx_sb = ptp.tile([128, CHUNK], F32, tag="pw", name="pw_sx")
                    nc.scalar.activation(sx_sb[:, :], psx[:, :], AF.Copy)
                    sy_sb = ptp.tile([128, CHUNK], F32, tag="pw", name="pw_sy")
                    nc.scalar.activation(sy_sb[:, :], psy[:, :], AF.Copy)
                    m1sq = ptp.tile([128, CHUNK], F32, tag="pw", name="pw_m1sq")
                    nc.scalar.activation(m1sq[0:M, :], pb5[0:M, :], AF.Square, scale=1.0 / 25.0)

                    # slope^2 (scaled 2^-12 via the Square scales) -> feat1
                    nc.vector.tensor_tensor(fsl(1), sx2[0:M, :], sy2[0:M, :], op=OP.add)
                    # curvature -> feat3
                    nc.vector.tensor_copy(fsl(3), pcv[0:M, :])
                    # rough^2 = relu(b5g2/25 - m1sq) -> feat4
                    r2 = ptp.tile([128, CHUNK], F32, tag="pw", name="pw_r2")
                    nc.vector.scalar_tensor_tensor(
                        out=r2[0:M, :], in0=pb52[0:M, :], scalar=1.0 / 25.0, in1=m1sq[0:M, :],
                        op0=OP.mult, op1=OP.subtract)
                    nc.vector.tensor_scalar(out=fsl(4), in0=r2[0:M, :],
                                            scalar1=0.0, scalar2=None, op0=OP.max)

                    # aspect = atan2(sy, sx) via octant fold:
                    #   a = min(|sx|,|sy|) / max(|sx|,|sy|, eps) in [0,1] (HW arctan domain)
                    #   t = swap? pi/2-base : base;  t = negx? pi-t : t;  asp = t*sign(sy)
                    ax = ptp.tile([128, CHUNK], F32, tag="pw", name="pw_ax")
                    nc.scalar.activation(ax[:, :], psx[:, :], AF.Abs)
                    ay = ptp.tile([128, CHUNK], F32, tag="pw", name="pw_ay")
                    nc.scalar.activation(ay[:, :], psy[:, :], AF.Abs)
                    amx = ptp.tile([128, CHUNK], F32, tag="pw", name="pw_amx")
                    nc.vector.scalar_tensor_tensor(out=amx[:, :], in0=ax[:, :],
                                                   scalar=EPS_RECIP, in1=ay[:, :],
                                                   op0=OP.max, op1=OP.max)
                    # amn = max(ax + ay - amx, 0) == min(ax, ay) (pool-legal ops only)
                    amn = ptp.tile([128, CHUNK], F32, tag="pw", name="pw_amn")
                    nc.gpsimd.tensor_tensor(amn[:, :], ax[:, :], ay[:, :], op=OP.add)
                    nc.gpsimd.tensor_tensor(amn[:, :], amn[:, :], amx[:, :], op=OP.subtract)
                    nc.gpsimd.tensor_scalar(out=amn[:, :], in0=amn[:, :],
                                            scalar1=0.0, scalar2=None, op0=OP.max)
                    rcp = ptp.tile([128, CHUNK], F32, tag="pw", name="pw_rcp")
                    rscr = ptp.tile([128, CHUNK], F32, tag="pw", name="pw_rscr")
                    nc.vector.reciprocal_approx_accurate(rcp[:, :], amx[:, :], rscr[:, :])
                    a_r = ptp.tile([128, CHUNK], F32, tag="pw", name="pw_a")
                    nc.vector.tensor_tensor(a_r[:, :], amn[:, :], rcp[:, :], op=OP.mult)
                    base = ptp.tile([128, CHUNK], F32, tag="pw", name="pw_base")
                    nc.scalar.activation(base[:, :], a_r[:, :], AF.Arctan)
                    swap = ptp.tile([128, CHUNK], F32, tag="pw", name="pw_swap")
                    nc.gpsimd.tensor_tensor(swap[:, :], ax[:, :], ay[:, :], op=OP.subtract)
                    nc.gpsimd.tensor_scalar(out=swap[:, :], in0=swap[:, :],
                                            scalar1=0.0, scalar2=None, op0=OP.is_lt)
                    # t = base + swap*(pi/2 - 2*base)
                    u1 = ptp.tile([128, CHUNK], F32, tag="pw", name="pw_u1")
                    nc.vector.tensor_scalar(out=u1[:, :], in0=base[:, :],
                                            scalar1=-2.0, scalar2=float(math.pi / 2),
                                            op0=OP.mult, op1=OP.add)
                    nc.gpsimd.tensor_tensor(u1[:, :], u1[:, :], swap[:, :], op=OP.mult)
                    t_r = ptp.tile([128, CHUNK], F32, tag="pw", name="pw_t")
                    nc.vector.tensor_tensor(t_r[:, :], u1[:, :], base[:, :], op=OP.add)
                    # t = t + negx*(pi - 2*t)
                    negx = ptp.tile([128, CHUNK], F32, tag="pw", name="pw_negx")
                    nc.gpsimd.tensor_scalar(out=negx[:, :], in0=sx_sb[:, :],
                                            scalar1=0.0, scalar2=None, op0=OP.is_lt)
                    u3 = ptp.tile([128, CHUNK], F32, tag="pw", name="pw_u3")
                    nc.vector.tensor_scalar(out=u3[:, :], in0=t_r[:, :],
                                            scalar1=-2.0, scalar2=float(math.pi),
                                            op0=OP.mult, op1=OP.add)
                    nc.gpsimd.tensor_tensor(u3[:, :], u3[:, :], negx[:, :], op=OP.mult)
                    nc.gpsimd.tensor_tensor(t_r[:, :], u3[:, :], t_r[:, :], op=OP.add)
                    # v = 1 - 2*[sy<0];  feat2 = t*v (+min accum)
                    v_r = ptp.tile([128, CHUNK], F32, tag="pw", name="pw_v")
                    nc.gpsimd.tensor_scalar(out=v_r[:, :], in0=sy_sb[:, :],
                                            scalar1=0.0, scalar2=None, op0=OP.is_lt)
                    nc.vector.tensor_scalar(out=v_r[:, :], in0=v_r[:, :],
                                            scalar1=-2.0, scalar2=1.0, op0=OP.mult, op1=OP.add)
                    nc.vector.tensor_tensor(fsl(2), t_r[0:M, :], v_r[0:M, :], op=OP.mult)


                # t1 reflect pads, then box15 stage B: ld = g - sum t1 shifts
                nc.vector.tensor_copy(t1_sb[:, 0:6], t1_sb[:, 12:6:-1])
                nc.vector.tensor_copy(t1_sb[:, 6 + NROWS:12 + NROWS],
                                      t1_sb[:, 4 + NROWS:NROWS - 2:-1])
                for c in range(NCHUNK):
                    ci = 2 * tidx + c
                    lo_t = 6 + c * CHUNK
                    lo = 7 + c * CHUNK
                    pld = psp.tile([128, CHUNK], F32, tag="ps")
                    for i, dy in enumerate((-6, -3, 0, 3, 6)):
                        nc.tensor.matmul(pld[:, :], band("bitm", s),
                                         t1_sb[:, lo_t + dy: lo_t + dy + CHUNK],
                                         start=(i == 0), stop=(i == 4))
                    nc.vector.scalar_tensor_tensor(
                        out=feats[5][0:M, fcol + c * CHUNK: fcol + (c + 1) * CHUNK],
                        in0=g_sb[po:po + M, lo: lo + CHUNK], scalar=0.0, in1=pld[0:M, :],
                        op0=OP.add, op1=OP.add)

                # per-tile min/max accumulation over the fp16 stores
                for f in range(6):
                    nc.vector.tensor_reduce(
                        accmn[f][0:M, tidx:tidx + 1], feats[f][0:M, fcol:fcol + 1024],
                        axis=mybir.AxisListType.X, op=OP.min)
                    nc.vector.tensor_reduce(
                        accmx[f][0:M, tidx:tidx + 1], feats[f][0:M, fcol:fcol + 1024],
                        axis=mybir.AxisListType.X, op=OP.max)

            # (accumulators now indexed by tile: col tidx in accmn/accmx)

            # ---------------- min/max finalize ----------------
            # coeff[:, 2f] = scale_f, coeff[:, 2f+1] = bias_f  (broadcast to all partitions)
            coeff = constp.tile([128, 16], F32)
            mnv = constp.tile([128, 8], F32)
            mxv = constp.tile([128, 8], F32)
            for f in range(6):
                # min: negate, all-reduce max, result in mnv[:, f] = -min
                nc.vector.tensor_reduce(mnv[0:128, f:f + 1], accmn[f][:],
                                        axis=mybir.AxisListType.X, op=OP.min)
                nc.vector.tensor_scalar(out=mnv[:, f:f + 1], in0=mnv[:, f:f + 1],
                                        scalar1=-1.0, scalar2=None, op0=OP.mult)
                nc.gpsimd.partition_all_reduce(mnv[:, f:f + 1], mnv[:, f:f + 1],
                                               channels=128, reduce_op=bass_isa.ReduceOp.max)
                nc.vector.tensor_scalar(out=mnv[:, f:f + 1], in0=mnv[:, f:f + 1],
                                        scalar1=-1.0, scalar2=None, op0=OP.mult)
                nc.vector.tensor_reduce(mxv[0:128, f:f + 1], accmx[f][:],
                                        axis=mybir.AxisListType.X, op=OP.max)
                nc.gpsimd.partition_all_reduce(mxv[:, f:f + 1], mxv[:, f:f + 1],
                                               channels=128, reduce_op=bass_isa.ReduceOp.max)

            # sqrt-domain transform for f1 (slope^2 scaled) and f4 (rough^2):
            # normalized = (sqrt(v) - sqrt(mn)) / max-clamp(sqrt(mx)-sqrt(mn))
            for f in (1, 4):
                nc.scalar.activation(mnv[:, f:f + 1], mnv[:, f:f + 1], AF.Sqrt)
                nc.scalar.activation(mxv[:, f:f + 1], mxv[:, f:f + 1], AF.Sqrt)

            # denom/coeff computation. Reference: denom = max(mx-mn, 1e-8) in the
            # reference's own feature scale; our stored scales differ by a positive
            # factor sf (f1: 2^-6 since slope_ref = 2^6*sqrt(stored); f2: 2pi;
            # others: 1). max(d, eps)*c == max(d*c, eps*c):
            # rs_stored = sf_inv... we clamp in stored scale with eps_f = 1e-8*sf_stored_per_ref.
            sf_ref_over_stored = [1.0, 2.0 ** 6, 1.0 / (2.0 * math.pi), 1.0, 1.0, 1.0]
            for f in range(6):
                d = constp.tile([128, 1], F32, tag="dtmp", name="dtmp")
                nc.vector.tensor_tensor(d[:], mxv[:, f:f + 1], mnv[:, f:f + 1], op=OP.subtract)
                eps_stored = 1e-8 / sf_ref_over_stored[f]
                nc.vector.tensor_scalar(out=d[:], in0=d[:], scalar1=float(eps_stored),
                                        scalar2=None, op0=OP.max)
                nc.vector.reciprocal(coeff[:, 2 * f:2 * f + 1], d[:])
                # bias = -mn * rs
                nc.vector.tensor_tensor(coeff[:, 2 * f + 1:2 * f + 2], mnv[:, f:f + 1],
                                        coeff[:, 2 * f:2 * f + 1], op=OP.mult)
                nc.vector.tensor_scalar(out=coeff[:, 2 * f + 1:2 * f + 2],
                                        in0=coeff[:, 2 * f + 1:2 * f + 2],
                                        scalar1=-1.0, scalar2=None, op0=OP.mult)

            # data-dependent 1.0 (forces phase-2 Sqrt after finalize -> no
            # sqrt/arctan ACT-table thrash inside phase 1)
            nc.vector.tensor_scalar(out=coeff[:, 12:13], in0=mnv[:, 0:1],
                                    scalar1=0.0, scalar2=1.0, op0=OP.mult, op1=OP.add)

            # ---------------- phase 2: normalize + store ----------------
            for tidx, (in_c0, K, out_c0, M) in enumerate(TILES):
                fcol = 1024 * tidx
                for f in range(6):
                    for h in range(2):
                        st = stagep.tile([128, CHUNK], F32, tag="stage", name="st")
                        src = feats[f][0:M, fcol + h * CHUNK:fcol + (h + 1) * CHUNK]
                        if f in (1, 4):
                            tmp = stagep.tile([128, CHUNK], F32, tag="sq", name="sqt")
                            nc.scalar.activation(tmp[0:M, :], src, AF.Sqrt,
                                                 scale=coeff[0:M, 12:13])
                            nc.scalar.activation(st[0:M, :], tmp[0:M, :], AF.Identity,
                                                 bias=coeff[0:M, 2 * f + 1:2 * f + 2],
                                                 scale=coeff[0:M, 2 * f:2 * f + 1])
                        else:
                            nc.vector.scalar_tensor_tensor(
                                out=st[0:M, :], in0=src, scalar=coeff[0:M, 2 * f:2 * f + 1],
                                in1=coeff[0:M, 2 * f + 1:2 * f + 2].broadcast_to((M, CHUNK)),
                                op0=OP.mult, op1=OP.add)
                        nc.sync.dma_start(out_d[f, out_c0:out_c0 + M, h * CHUNK:(h + 1) * CHUNK],
                                          st[0:M, :])

    nc.compile()
    return nc


def _host_gray(x):
    """Replicate reference fp32 arithmetic: scale then per-channel weight, summed."""
    x = np.asarray(x, dtype=np.float32)
    if float(np.max(x)) <= 1.0:
        x = x * np.float32(255.0)
    w = np.asarray([0.299, 0.587, 0.114], np.float32)
    gray = (x[:, 0] * w[0] + x[:, 1] * w[1]) + x[:, 2] * w[2]
    return gray  # [B, H, W] fp32


def kernel(x):
    x = np.asarray(x)
    B = x.shape[0]
    assert x.shape == (8, 3, 1024, 1024)
    gray = _host_gray(x)  # [8, H, W]
    grayT = np.ascontiguousarray(np.transpose(gray, (0, 2, 1)))  # [8, cols, rows]

    if "nc" not in _NC_CACHE:
        _NC_CACHE["nc"] = build_nc()
        _NC_CACHE["bands"] = build_bands()[0]
    nc = _NC_CACHE["nc"]
    bands_np = _NC_CACHE["bands"]

    in_maps = [{"gray": grayT[i], "bands": bands_np} for i in range(B)]
    res = bass_utils.run_bass_kernel_spmd(nc, in_maps, core_ids=list(range(8)))
    out = np.stack([res.results[i]["out"] for i in range(B)])  # [8, 6, cols, rows]
    return np.ascontiguousarray(np.transpose(out, (0, 1, 3, 2)).astype(np.float32))

